# revision 24
# baseline (speedup 1.0000x reference)
"""Trainium2 Bass kernel for nn_Head_75118978007668.

Computes, for x:[B,S,D], concept_map(cm):[D,D,D] (B=4, S=2048, D=128):
    s[b,t] = sum_{j<t} lam^(t-j) x[b,j]          (lam = 1/1.2 decayed prefix sum)
    out[b,t,f] = sum_{d,e} x[b,t,d] * s[b,t,e] * cm[f,d,e]

Sharding: 8 cores, each owns 1024 contiguous positions of one batch row
(4 rows x 2 halves).  The scan carry across the half-split is recovered
exactly (to fp32) from a 256-position halo, since lam^256 ~ 4.5e-21 is far
below fp32 resolution.

Per-core dataflow (positions tiled 8 x 128):
  - carries: small PE matmuls build s(tile_start) for all 8 tiles at once
  - s tiles: triangular matmul  s = L @ x_tile + pow (x) carry   (PE, fp32)
  - main:    Y[p, (e,f)] = xT_tile.T @ W2   (PE, bf16 in / fp32 psum out)
    then the weighted e-reduction acc[p,f] += s[p,e] * Y[p,(e,f)] is split
    across three engine streams (per tile of 128 e-values):
      A (56 e): DVE scalar_tensor_tensor directly from PSUM, 2 alternating
        accumulators to keep the dependency chain off the critical path.
      B (40 e): Act (scalar engine) scaled-copies z_e = s_e*Y_e into bf16
        SBUF columns; DVE folds the 40 columns with a wide bf16 add-tree.
      C (32 e): Act copies Y chunks PSUM->SBUF; GpSimd does the STT there
        (GpSimd has no PSUM port).
  where W2[d, e*128+f] = cm[f, d, e]  (host-transposed, bf16).
"""

import numpy as np
import ml_dtypes

import concourse.bass as bass
import concourse.tile as tile
from concourse import bacc, mybir
from concourse.bass import ds, ts
from concourse.bass_utils import run_bass_kernel_spmd

B, S, D = 4, 2048, 128
NCORES = 8
CHUNK = S // 2          # positions per core (1024)
NT = CHUNK // 128       # position tiles per core (8)
P = 128
HALO = 256
F32 = mybir.dt.float32
BF16 = mybir.dt.bfloat16

# match the reference's fp32 constant 1.2 exactly
LAM = 1.0 / np.float64(np.float32(1.2))

# Every e produces one bf16 z column z_e = s_e * Y_e (z col index == e);
# a 7-level bf16 add-tree on DVE folds all 128 columns into the output.
# Producers per tile, by chunk kind (chunk = 4 consecutive e, f minor):
#   'C': Act copies Y chunk PSUM->SBUF, GpSimd broadcast-mult 4 e per op
#        (GpSimd has no PSUM port, ~3.3ns/elem, stream-bound)
#   'D' (consecutive pairs): one DVE broadcast-mult over a [P,8,128] 2-bank
#        PSUM mega tile = 8 e per op (amortizes DVE fixed cost)
#   'B': Act per-e scaled-copy straight from PSUM
# Mix tuned from engine-busy traces: DVE ~19us/tile (7 megas + tree),
# Act ~18us (16 z + 14 copies), GpSimd ~19us (14 chunks).
GROUP_KINDS = ["DDCB"] * 6 + ["DDCC"] * 2
ZB = 128

_CACHE = {}
LAST_RESULTS = None


def _host_constants():
    k = np.arange(P, dtype=np.float64)
    i = k
    # LT[i, k] = L[k, i] = lam^(k-i) for i < k   (lhsT of the triangular scan)
    LT = np.where(i[:, None] < k[None, :], LAM ** (k[None, :] - i[:, None]), 0.0)
    powv = (LAM ** k)[None, :]                      # [1, 128]
    vw = (LAM ** (P - i))[:, None]                  # [128, 1]
    j = np.arange(HALO, dtype=np.float64)           # halo weights lam^(256-j)
    hw = (LAM ** (HALO - j)).reshape(2, P).T        # [128, 2]  hw[i, u] = lam^(256-(u*128+i))
    # M9[t, jj]: c_t = sum_jj M9[t, jj] * V9[jj];  V9 = [c0, v_0..v_7]
    t = np.arange(NT, dtype=np.float64)
    M9 = np.zeros((NT, NT + 1), dtype=np.float64)
    M9[:, 0] = LAM ** (P * t)
    for tt in range(NT):
        for jj in range(tt):
            M9[tt, jj + 1] = LAM ** (P * (tt - 1 - jj))
    LT9 = M9.T                                      # [9, 8]
    f32 = np.float32
    return {
        "lt": LT.astype(f32),
        "powv": powv.astype(f32),
        "vw": vw.astype(f32),
        "hw": hw.astype(f32),
        "lt9": LT9.astype(f32),
    }


def _build_nc():
    nc = bacc.Bacc("TRN2", target_bir_lowering=False, debug=False,
                   num_devices=NCORES)
    x_d = nc.declare_dram_parameter("x", [P, NT, P], F32, isOutput=False)        # [i, t, e]
    xt_d = nc.declare_dram_parameter("xt", [P, CHUNK], BF16, isOutput=False)     # [d, p]
    halo_d = nc.declare_dram_parameter("halo", [P, 2, P], F32, isOutput=False)   # [i, u, e]
    w2_d = nc.declare_dram_parameter("w2", [P, P * P], BF16, isOutput=False)     # [d, (e,f)]
    lt_d = nc.declare_dram_parameter("lt", [P, P], F32, isOutput=False)
    pow_d = nc.declare_dram_parameter("powv", [1, P], F32, isOutput=False)
    vw_d = nc.declare_dram_parameter("vw", [P, 1], F32, isOutput=False)
    hw_d = nc.declare_dram_parameter("hw", [P, 2], F32, isOutput=False)
    lt9_d = nc.declare_dram_parameter("lt9", [NT + 1, NT], F32, isOutput=False)
    out_d = nc.declare_dram_parameter("out", [P, NT, P], F32, isOutput=True)  # [p, t, f]

    mult = mybir.AluOpType.mult
    add = mybir.AluOpType.add

    with tile.TileContext(nc) as tc:
        with tc.tile_pool(name="consts", bufs=1) as consts:
            w2_sb = [consts.tile([P, 2048], BF16, name=f"w2_sb{i}")
                     for i in range(8)]
            xt_sb = consts.tile([P, CHUNK], BF16)
            x_sb = consts.tile([P, NT, P], F32)
            halo_sb = consts.tile([P, 2, P], F32)
            lt_sb = consts.tile([P, P], F32)
            pow_sb = consts.tile([1, P], F32)
            vw_sb = consts.tile([P, 1], F32)
            hw_sb = consts.tile([P, 2], F32)
            lt9_sb = consts.tile([NT + 1, NT], F32)
            v9_sb = consts.tile([NT + 1, P], F32)
            c0_sb = consts.tile([1, P], F32)
            va_sb = consts.tile([1, 4 * P], F32)
            vb_sb = consts.tile([1, 4 * P], F32)
            c8_sb = consts.tile([NT, P], F32)
            c_all = consts.tile([1, NT * P], F32)    # [1, (t,e)] carries
            s_sb = consts.tile([P, NT, P], F32)      # [p, t, e]
            out_sb = consts.tile([P, NT, P], F32)    # [p, t, f]

            # small inputs first: the carry machinery (x, halo, consts) is on
            # the critical path to s_sb; the 4 MB w2 streams in behind it
            nc.sync.dma_start(out=x_sb[:, :, :], in_=x_d[:, :, :])
            nc.sync.dma_start(out=halo_sb[:, :, :], in_=halo_d[:, :, :])
            nc.sync.dma_start(out=lt_sb[:, :], in_=lt_d[:, :])
            nc.sync.dma_start(out=pow_sb[:, :], in_=pow_d[:, :])
            nc.sync.dma_start(out=vw_sb[:, :], in_=vw_d[:, :])
            nc.sync.dma_start(out=hw_sb[:, :], in_=hw_d[:, :])
            nc.sync.dma_start(out=lt9_sb[:, :], in_=lt9_d[:, :])
            nc.sync.dma_start(out=xt_sb[:, :], in_=xt_d[:, :])

            # ---- carries: c_t = s[tile_start t] for all 8 tiles ----
            with tc.tile_pool(name="psum_c", bufs=1, space="PSUM") as psum_c:
                c0_ps = psum_c.tile([1, P], F32)
                nc.tensor.matmul(c0_ps[:, :], lhsT=hw_sb[:, 0:1],
                                 rhs=halo_sb[:, 0, :], start=True, stop=False)
                nc.tensor.matmul(c0_ps[:, :], lhsT=hw_sb[:, 1:2],
                                 rhs=halo_sb[:, 1, :], start=False, stop=True)
                vps_a = psum_c.tile([1, 4 * P], F32, tag="vps_a")
                vps_b = psum_c.tile([1, 4 * P], F32, tag="vps_b")
                nc.tensor.matmul(vps_a[:, :], lhsT=vw_sb[:, :],
                                 rhs=x_sb[:, 0:4, :], start=True, stop=True)
                nc.tensor.matmul(vps_b[:, :], lhsT=vw_sb[:, :],
                                 rhs=x_sb[:, 4:8, :], start=True, stop=True)
                nc.vector.tensor_copy(c0_sb[:, :], c0_ps[:, :])
                nc.vector.tensor_copy(va_sb[:, :], vps_a[:, :])
                nc.vector.tensor_copy(vb_sb[:, :], vps_b[:, :])
                nc.sync.dma_start(out=v9_sb[0:1, :], in_=c0_sb[:, :])
                nc.sync.dma_start(out=v9_sb[1:5, :], in_=va_sb[:, :])
                nc.sync.dma_start(out=v9_sb[5:9, :], in_=vb_sb[:, :])
                c_ps = psum_c.tile([NT, P], F32, tag="c_ps")
                nc.tensor.matmul(c_ps[:, :], lhsT=lt9_sb[:, :],
                                 rhs=v9_sb[:, :], start=True, stop=True)
                nc.vector.tensor_copy(c8_sb[:, :], c_ps[:, :])
                nc.sync.dma_start(out=c_all[:, :], in_=c8_sb[:, :])

            # w2 now: its first segment lands well before the first
            # main-loop matmul needs it, and its descriptors no longer
            # delay the carry chain's small SBUF DMAs above
            for i in range(8):
                nc.sync.dma_start(out=w2_sb[i][:, :],
                                  in_=w2_d[:, ds(2048 * i, 2048)])

            # ---- s tiles: s = L @ x_t + pow (x) c_t ----
            with tc.tile_pool(name="psum_s", bufs=2, space="PSUM") as psum_s:
                for t in range(NT):
                    sp = psum_s.tile([P, P], F32)
                    nc.tensor.matmul(sp[:, :], lhsT=lt_sb[:, :],
                                     rhs=x_sb[:, t, :], start=True, stop=False)
                    c_rhs = c0_sb[:, :] if t == 0 else c_all[:, ts(t, P)]
                    nc.tensor.matmul(sp[:, :], lhsT=pow_sb[:, :],
                                     rhs=c_rhs, start=False, stop=True)
                    nc.vector.tensor_copy(s_sb[:, t, :], sp[:, :])

            # ---- main: Y = xT_t.T @ W2 chunks; 3-stream weighted e-reduce ----
            with tc.tile_pool(name="psum_y", bufs=4, space="PSUM") as psum_y, \
                 tc.tile_pool(name="psum_m", bufs=2, space="PSUM") as psum_m, \
                 tc.tile_pool(name="zpool", bufs=4) as zpool, \
                 tc.tile_pool(name="ypool", bufs=6) as ypool:
                def tree_ops(t, zb):
                    # fold 128 bf16 z columns: 7 halving levels, last into
                    # f32, then DMA out -- as a list of thunks so the ops can
                    # be interleaved between the next tile's mega-mults
                    # (keeps DVE consuming psum_m so the PE never stalls)
                    ops = []
                    for half in (64, 32, 16, 8, 4, 2):
                        ops.append(lambda h=half: nc.vector.tensor_tensor(
                            zb[:, 0:h, :], zb[:, 0:h, :],
                            zb[:, h:2 * h, :], add))

                    def last():
                        nc.vector.tensor_tensor(out_sb[:, t, :], zb[:, 0, :],
                                                zb[:, 1, :], add)
                        nc.sync.dma_start(out=out_d[:, t, :],
                                          in_=out_sb[:, t, :])
                    ops.append(last)
                    return ops

                def d_group(t, zb, g):
                    # the 'DD' pair of group g: DVE 8-e mega mult
                    xt_t = xt_sb[:, ts(t, P)]
                    i = GROUP_KINDS[g].index("D")
                    c = 4 * g + i
                    e0 = 4 * c
                    mp = psum_m.tile([P, 8, P], F32)
                    for h in range(2):
                        ch = c + h
                        nc.tensor.matmul(
                            mp[:, 4 * h:4 * h + 4, :], lhsT=xt_t,
                            rhs=w2_sb[ch // 4][:, ds(512 * (ch % 4), 512)],
                            start=True, stop=True)
                    nc.vector.tensor_tensor(
                        zb[:, e0:e0 + 8, :], mp[:, :, :],
                        s_sb[:, t, e0:e0 + 8, None].to_broadcast([P, 8, P]),
                        mult)

                def cb_group(t, zb, g):
                    # the 'C'/'B' chunks of group g
                    xt_t = xt_sb[:, ts(t, P)]
                    for i, kind in enumerate(GROUP_KINDS[g]):
                        if kind == "D":
                            continue
                        c = 4 * g + i
                        e0 = 4 * c
                        yp = psum_y.tile([P, 512], F32)
                        nc.tensor.matmul(
                            yp[:, :], lhsT=xt_t,
                            rhs=w2_sb[c // 4][:, ds(512 * (c % 4), 512)],
                            start=True, stop=True)
                        if kind == "C":
                            ysb = ypool.tile([P, 4, P], F32)
                            nc.scalar.copy(ysb[:, :, :], yp[:, :])
                            nc.gpsimd.tensor_tensor(
                                zb[:, e0:e0 + 4, :], ysb[:, :, :],
                                s_sb[:, t, e0:e0 + 4, None].to_broadcast(
                                    [P, 4, P]),
                                mult)
                        else:  # 'B'
                            for jj in range(4):
                                e = e0 + jj
                                nc.scalar.mul(zb[:, e, :], yp[:, ts(jj, P)],
                                              s_sb[:, t, e:e + 1])

                # D-stream (PE pair-matmuls + DVE megas) runs one tile AHEAD
                # of the C/B streams, so the final tile leaves only Act and
                # GpSimd producer work overlapping the last trees on DVE.
                zbs = [None] * NT
                for tt in range(2):
                    zbs[tt] = zpool.tile([P, ZB, P], BF16, name="zb")
                    for g in range(8):
                        d_group(tt, zbs[tt], g)
                pending = []
                for t in range(NT):
                    if t + 2 < NT:
                        zbs[t + 2] = zpool.tile([P, ZB, P], BF16,
                                                name="zb")
                    for g in range(8):
                        if pending:
                            pending.pop(0)()
                        if t + 2 < NT:
                            d_group(t + 2, zbs[t + 2], g)
                        cb_group(t, zbs[t], g)
                    for op in pending:
                        op()
                    pending = tree_ops(t, zbs[t])
                    zbs[t - 1] = None
                for op in pending:
                    op()
    nc.finalize()
    return nc


def _get_nc():
    if "nc" not in _CACHE:
        _CACHE["nc"] = _build_nc()
    return _CACHE["nc"]


def kernel(x, concept_map, _trace=False):
    global LAST_RESULTS
    x = np.asarray(x, dtype=np.float32)
    cm = np.asarray(concept_map, dtype=np.float32)
    assert x.shape == (B, S, D) and cm.shape == (D, D, D)

    consts = _host_constants()
    # W2[d, e*128+f] = cm[f, d, e]
    w2 = np.ascontiguousarray(
        np.transpose(cm, (1, 2, 0)).reshape(D, D * D)).astype(ml_dtypes.bfloat16)

    in_maps = []
    for core in range(NCORES):
        b, half = divmod(core, 2)
        lo = half * CHUNK
        xc = x[b, lo:lo + CHUNK]                          # [1024, 128]
        # [i, t, e] interleaved layout (partition = within-tile position)
        x_il = np.ascontiguousarray(
            xc.reshape(NT, P, D).transpose(1, 0, 2))
        xt = np.ascontiguousarray(xc.T).astype(ml_dtypes.bfloat16)  # [d, p]
        if half == 0:
            halo = np.zeros((P, 2, D), dtype=np.float32)
        else:
            h = x[b, lo - HALO:lo]                        # [256, 128]
            halo = np.ascontiguousarray(h.reshape(2, P, D).transpose(1, 0, 2))
        in_maps.append({
            "x": x_il, "xt": xt, "halo": halo, "w2": w2, **consts,
        })

    nc = _get_nc()
    res = run_bass_kernel_spmd(nc, in_maps, list(range(NCORES)), trace=_trace)
    LAST_RESULTS = res

    out = np.empty((B, S, D), dtype=np.float32)
    for core in range(NCORES):
        b, half = divmod(core, 2)
        o = res.results[core]["out"]                      # [p, t, f]
        out[b, half * CHUNK:(half + 1) * CHUNK] = (
            o.transpose(1, 0, 2).reshape(CHUNK, D))
    return out


# revision 25
# speedup vs baseline: 1.1666x; 1.1666x over previous
"""Trainium2 Bass kernel for nn_Head_75118978007668.

Computes, for x:[B,S,D], concept_map(cm):[D,D,D] (B=4, S=2048, D=128):
    s[b,t] = sum_{j<t} lam^(t-j) x[b,j]          (lam = 1/1.2 decayed prefix sum)
    out[b,t,f] = sum_{d,e} x[b,t,d] * s[b,t,e] * cm[f,d,e]

Sharding: 8 cores, each owns 1024 contiguous positions of one batch row
(4 rows x 2 halves).  The scan carry across the half-split is recovered
exactly (to fp32) from a 256-position halo, since lam^256 ~ 4.5e-21 is far
below fp32 resolution.

Per-core dataflow (positions tiled 8 x 128):
  - carries: small PE matmuls build s(tile_start) for all 8 tiles at once
  - s tiles: triangular matmul  s = L @ x_tile + pow (x) carry   (PE, fp32)
  - main:    Y[p, (e,f)] = xT_tile.T @ W2   (PE, bf16 in / fp32 psum out)
    then the weighted e-reduction acc[p,f] += s[p,e] * Y[p,(e,f)] is split
    across three engine streams (per tile of 128 e-values):
      A (56 e): DVE scalar_tensor_tensor directly from PSUM, 2 alternating
        accumulators to keep the dependency chain off the critical path.
      B (40 e): Act (scalar engine) scaled-copies z_e = s_e*Y_e into bf16
        SBUF columns; DVE folds the 40 columns with a wide bf16 add-tree.
      C (32 e): Act copies Y chunks PSUM->SBUF; GpSimd does the STT there
        (GpSimd has no PSUM port).
  where W2[d, e*128+f] = cm[f, d, e]  (host-transposed, bf16).
"""

import numpy as np
import ml_dtypes

import concourse.bass as bass
import concourse.tile as tile
from concourse import bacc, mybir
from concourse.bass import ds, ts
from concourse.bass_utils import run_bass_kernel_spmd

B, S, D = 4, 2048, 128
NCORES = 8
CHUNK = S // 2          # positions per core (1024)
NT = CHUNK // 128       # position tiles per core (8)
P = 128
HALO = 256
F32 = mybir.dt.float32
BF16 = mybir.dt.bfloat16

# match the reference's fp32 constant 1.2 exactly
LAM = 1.0 / np.float64(np.float32(1.2))

# Every e produces one bf16 z column z_e = s_e * Y_e (z col index == e);
# a 7-level bf16 add-tree on DVE folds all 128 columns into the output.
# Producers per tile, by chunk kind (chunk = 4 consecutive e, f minor):
#   'C': Act copies Y chunk PSUM->SBUF, GpSimd broadcast-mult 4 e per op
#        (GpSimd has no PSUM port, ~3.3ns/elem, stream-bound)
#   'D' (consecutive pairs): one DVE broadcast-mult over a [P,8,128] 2-bank
#        PSUM mega tile = 8 e per op (amortizes DVE fixed cost)
#   'B': Act per-e scaled-copy straight from PSUM
# Mix tuned from engine-busy traces: DVE ~19us/tile (7 megas + tree),
# Act ~18us (16 z + 14 copies), GpSimd ~19us (14 chunks).
GROUP_KINDS = ["DDCB"] * 6 + ["DDCC"] * 2
ZB = 128

_CACHE = {}
LAST_RESULTS = None


def _host_constants():
    k = np.arange(P, dtype=np.float64)
    i = k
    # LT[i, k] = L[k, i] = lam^(k-i) for i < k   (lhsT of the triangular scan)
    LT = np.where(i[:, None] < k[None, :], LAM ** (k[None, :] - i[:, None]), 0.0)
    powv = (LAM ** k)[None, :]                      # [1, 128]
    vw = (LAM ** (P - i))[:, None]                  # [128, 1]
    j = np.arange(HALO, dtype=np.float64)           # halo weights lam^(256-j)
    hw = (LAM ** (HALO - j)).reshape(2, P).T        # [128, 2]  hw[i, u] = lam^(256-(u*128+i))
    # M9[t, jj]: c_t = sum_jj M9[t, jj] * V9[jj];  V9 = [c0, v_0..v_7]
    t = np.arange(NT, dtype=np.float64)
    M9 = np.zeros((NT, NT + 1), dtype=np.float64)
    M9[:, 0] = LAM ** (P * t)
    for tt in range(NT):
        for jj in range(tt):
            M9[tt, jj + 1] = LAM ** (P * (tt - 1 - jj))
    LT9 = M9.T                                      # [9, 8]
    f32 = np.float32
    return {
        "lt": LT.astype(f32),
        "powv": powv.astype(f32),
        "vw": vw.astype(f32),
        "hw": hw.astype(f32),
        "lt9": LT9.astype(f32),
    }


def _build_nc():
    nc = bacc.Bacc("TRN2", target_bir_lowering=False, debug=False,
                   num_devices=NCORES)
    x_d = nc.declare_dram_parameter("x", [P, NT, P], F32, isOutput=False)        # [i, t, e]
    xt_d = nc.declare_dram_parameter("xt", [P, CHUNK], BF16, isOutput=False)     # [d, p]
    halo_d = nc.declare_dram_parameter("halo", [P, 2, P], F32, isOutput=False)   # [i, u, e]
    w2_d = nc.declare_dram_parameter("w2", [P, P * P], BF16, isOutput=False)     # [d, (e,f)]
    lt_d = nc.declare_dram_parameter("lt", [P, P], F32, isOutput=False)
    pow_d = nc.declare_dram_parameter("powv", [1, P], F32, isOutput=False)
    vw_d = nc.declare_dram_parameter("vw", [P, 1], F32, isOutput=False)
    hw_d = nc.declare_dram_parameter("hw", [P, 2], F32, isOutput=False)
    lt9_d = nc.declare_dram_parameter("lt9", [NT + 1, NT], F32, isOutput=False)
    out_d = nc.declare_dram_parameter("out", [P, NT, P], F32, isOutput=True)  # [p, t, f]

    mult = mybir.AluOpType.mult
    add = mybir.AluOpType.add

    with tile.TileContext(nc) as tc:
        with tc.tile_pool(name="consts", bufs=1) as consts:
            w2_sb = [consts.tile([P, 2048], BF16, name=f"w2_sb{i}")
                     for i in range(8)]
            xt_sb = consts.tile([P, CHUNK], BF16)
            x_sb = consts.tile([P, NT, P], F32)
            halo_sb = consts.tile([P, 2, P], F32)
            lt_sb = consts.tile([P, P], F32)
            pow_sb = consts.tile([1, P], F32)
            vw_sb = consts.tile([P, 1], F32)
            hw_sb = consts.tile([P, 2], F32)
            lt9_sb = consts.tile([NT + 1, NT], F32)
            v9_sb = consts.tile([NT + 1, P], F32)
            c0_sb = consts.tile([1, P], F32)
            va_sb = consts.tile([1, 4 * P], F32)
            vb_sb = consts.tile([1, 4 * P], F32)
            c8_sb = consts.tile([NT, P], F32)
            c_all = consts.tile([1, NT * P], F32)    # [1, (t,e)] carries
            s_sb = consts.tile([P, NT, P], F32)      # [p, t, e]
            out_sb = consts.tile([P, NT, P], F32)    # [p, t, f]

            # small inputs first: the carry machinery (x, halo, consts) is on
            # the critical path to s_sb; the 4 MB w2 streams in behind it
            nc.sync.dma_start(out=x_sb[:, :, :], in_=x_d[:, :, :])
            nc.sync.dma_start(out=halo_sb[:, :, :], in_=halo_d[:, :, :])
            nc.sync.dma_start(out=lt_sb[:, :], in_=lt_d[:, :])
            nc.sync.dma_start(out=pow_sb[:, :], in_=pow_d[:, :])
            nc.sync.dma_start(out=vw_sb[:, :], in_=vw_d[:, :])
            nc.sync.dma_start(out=hw_sb[:, :], in_=hw_d[:, :])
            nc.sync.dma_start(out=lt9_sb[:, :], in_=lt9_d[:, :])
            nc.sync.dma_start(out=xt_sb[:, :], in_=xt_d[:, :])

            # ---- carries: c_t = s[tile_start t] for all 8 tiles ----
            with tc.tile_pool(name="psum_c", bufs=1, space="PSUM") as psum_c:
                c0_ps = psum_c.tile([1, P], F32)
                nc.tensor.matmul(c0_ps[:, :], lhsT=hw_sb[:, 0:1],
                                 rhs=halo_sb[:, 0, :], start=True, stop=False)
                nc.tensor.matmul(c0_ps[:, :], lhsT=hw_sb[:, 1:2],
                                 rhs=halo_sb[:, 1, :], start=False, stop=True)
                vps_a = psum_c.tile([1, 4 * P], F32, tag="vps_a")
                vps_b = psum_c.tile([1, 4 * P], F32, tag="vps_b")
                nc.tensor.matmul(vps_a[:, :], lhsT=vw_sb[:, :],
                                 rhs=x_sb[:, 0:4, :], start=True, stop=True)
                nc.tensor.matmul(vps_b[:, :], lhsT=vw_sb[:, :],
                                 rhs=x_sb[:, 4:8, :], start=True, stop=True)
                nc.vector.tensor_copy(c0_sb[:, :], c0_ps[:, :])
                nc.vector.tensor_copy(va_sb[:, :], vps_a[:, :])
                nc.vector.tensor_copy(vb_sb[:, :], vps_b[:, :])
                nc.sync.dma_start(out=v9_sb[0:1, :], in_=c0_sb[:, :])
                nc.sync.dma_start(out=v9_sb[1:5, :], in_=va_sb[:, :])
                nc.sync.dma_start(out=v9_sb[5:9, :], in_=vb_sb[:, :])
                c_ps = psum_c.tile([NT, P], F32, tag="c_ps")
                nc.tensor.matmul(c_ps[:, :], lhsT=lt9_sb[:, :],
                                 rhs=v9_sb[:, :], start=True, stop=True)
                nc.vector.tensor_copy(c8_sb[:, :], c_ps[:, :])
                nc.sync.dma_start(out=c_all[:, :], in_=c8_sb[:, :])

            # w2 now: its first segment lands well before the first
            # main-loop matmul needs it, and its descriptors no longer
            # delay the carry chain's small SBUF DMAs above
            for i in range(8):
                nc.sync.dma_start(out=w2_sb[i][:, :],
                                  in_=w2_d[:, ds(2048 * i, 2048)])

            # ---- s tiles: s = L @ x_t + pow (x) c_t ----
            with tc.tile_pool(name="psum_s", bufs=2, space="PSUM") as psum_s:
                for t in range(NT):
                    sp = psum_s.tile([P, P], F32)
                    nc.tensor.matmul(sp[:, :], lhsT=lt_sb[:, :],
                                     rhs=x_sb[:, t, :], start=True, stop=False)
                    c_rhs = c0_sb[:, :] if t == 0 else c_all[:, ts(t, P)]
                    nc.tensor.matmul(sp[:, :], lhsT=pow_sb[:, :],
                                     rhs=c_rhs, start=False, stop=True)
                    nc.vector.tensor_copy(s_sb[:, t, :], sp[:, :])

            # ---- main: Y = xT_t.T @ W2 chunks; 3-stream weighted e-reduce ----
            with tc.tile_pool(name="psum_y", bufs=4, space="PSUM") as psum_y, \
                 tc.tile_pool(name="psum_m", bufs=2, space="PSUM") as psum_m, \
                 tc.tile_pool(name="zpool", bufs=3) as zpool, \
                 tc.tile_pool(name="ypool", bufs=6) as ypool:
                def tree_ops(t, zb):
                    # fold 128 bf16 z columns: 7 halving levels, last into
                    # f32, then DMA out -- as a list of thunks so the ops can
                    # be interleaved between the next tile's mega-mults
                    # (keeps DVE consuming psum_m so the PE never stalls)
                    ops = []
                    for half in (64, 32, 16, 8, 4, 2):
                        ops.append(lambda h=half: nc.vector.tensor_tensor(
                            zb[:, 0:h, :], zb[:, 0:h, :],
                            zb[:, h:2 * h, :], add))

                    def last():
                        nc.vector.tensor_tensor(out_sb[:, t, :], zb[:, 0, :],
                                                zb[:, 1, :], add)
                        nc.sync.dma_start(out=out_d[:, t, :],
                                          in_=out_sb[:, t, :])
                    ops.append(last)
                    return ops

                def d_group(t, zb, g):
                    # the 'DD' pair of group g: DVE 8-e mega mult
                    xt_t = xt_sb[:, ts(t, P)]
                    i = GROUP_KINDS[g].index("D")
                    c = 4 * g + i
                    e0 = 4 * c
                    mp = psum_m.tile([P, 8, P], F32)
                    for h in range(2):
                        ch = c + h
                        nc.tensor.matmul(
                            mp[:, 4 * h:4 * h + 4, :], lhsT=xt_t,
                            rhs=w2_sb[ch // 4][:, ds(512 * (ch % 4), 512)],
                            start=True, stop=True)
                    nc.vector.tensor_tensor(
                        zb[:, e0:e0 + 8, :], mp[:, :, :],
                        s_sb[:, t, e0:e0 + 8, None].to_broadcast([P, 8, P]),
                        mult)

                def cb_group(t, zb, g):
                    # the 'C'/'B' chunks of group g
                    xt_t = xt_sb[:, ts(t, P)]
                    for i, kind in enumerate(GROUP_KINDS[g]):
                        if kind == "D":
                            continue
                        c = 4 * g + i
                        e0 = 4 * c
                        yp = psum_y.tile([P, 512], F32)
                        nc.tensor.matmul(
                            yp[:, :], lhsT=xt_t,
                            rhs=w2_sb[c // 4][:, ds(512 * (c % 4), 512)],
                            start=True, stop=True)
                        if kind == "C":
                            ysb = ypool.tile([P, 4, P], F32)
                            nc.scalar.copy(ysb[:, :, :], yp[:, :])
                            nc.gpsimd.tensor_tensor(
                                zb[:, e0:e0 + 4, :], ysb[:, :, :],
                                s_sb[:, t, e0:e0 + 4, None].to_broadcast(
                                    [P, 4, P]),
                                mult)
                        else:  # 'B'
                            for jj in range(4):
                                e = e0 + jj
                                nc.scalar.mul(zb[:, e, :], yp[:, ts(jj, P)],
                                              s_sb[:, t, e:e + 1])

                # D-stream (PE pair-matmuls + DVE megas) runs one tile AHEAD
                # of the C/B streams, so the final tile leaves only Act and
                # GpSimd producer work overlapping the last trees on DVE.
                zbs = [None] * NT
                zbs[0] = zpool.tile([P, ZB, P], BF16, name="zb")
                for g in range(8):
                    d_group(0, zbs[0], g)
                pending = []
                for t in range(NT):
                    if t + 1 < NT:
                        zbs[t + 1] = zpool.tile([P, ZB, P], BF16,
                                                name="zb")
                    for g in range(8):
                        if pending:
                            pending.pop(0)()
                        if t + 1 < NT:
                            d_group(t + 1, zbs[t + 1], g)
                        cb_group(t, zbs[t], g)
                    for op in pending:
                        op()
                    pending = tree_ops(t, zbs[t])
                    zbs[t - 1] = None
                for op in pending:
                    op()
    nc.finalize()
    return nc


def _get_nc():
    if "nc" not in _CACHE:
        _CACHE["nc"] = _build_nc()
    return _CACHE["nc"]


def kernel(x, concept_map, _trace=False):
    global LAST_RESULTS
    x = np.asarray(x, dtype=np.float32)
    cm = np.asarray(concept_map, dtype=np.float32)
    assert x.shape == (B, S, D) and cm.shape == (D, D, D)

    consts = _host_constants()
    # W2[d, e*128+f] = cm[f, d, e]
    w2 = np.ascontiguousarray(
        np.transpose(cm, (1, 2, 0)).reshape(D, D * D)).astype(ml_dtypes.bfloat16)

    in_maps = []
    for core in range(NCORES):
        b, half = divmod(core, 2)
        lo = half * CHUNK
        xc = x[b, lo:lo + CHUNK]                          # [1024, 128]
        # [i, t, e] interleaved layout (partition = within-tile position)
        x_il = np.ascontiguousarray(
            xc.reshape(NT, P, D).transpose(1, 0, 2))
        xt = np.ascontiguousarray(xc.T).astype(ml_dtypes.bfloat16)  # [d, p]
        if half == 0:
            halo = np.zeros((P, 2, D), dtype=np.float32)
        else:
            h = x[b, lo - HALO:lo]                        # [256, 128]
            halo = np.ascontiguousarray(h.reshape(2, P, D).transpose(1, 0, 2))
        in_maps.append({
            "x": x_il, "xt": xt, "halo": halo, "w2": w2, **consts,
        })

    nc = _get_nc()
    res = run_bass_kernel_spmd(nc, in_maps, list(range(NCORES)), trace=_trace)
    LAST_RESULTS = res

    out = np.empty((B, S, D), dtype=np.float32)
    for core in range(NCORES):
        b, half = divmod(core, 2)
        o = res.results[core]["out"]                      # [p, t, f]
        out[b, half * CHUNK:(half + 1) * CHUNK] = (
            o.transpose(1, 0, 2).reshape(CHUNK, D))
    return out


# revision 26
# speedup vs baseline: 1.1735x; 1.0059x over previous
"""Trainium2 Bass kernel for nn_Head_75118978007668.

Computes, for x:[B,S,D], concept_map(cm):[D,D,D] (B=4, S=2048, D=128):
    s[b,t] = sum_{j<t} lam^(t-j) x[b,j]          (lam = 1/1.2 decayed prefix sum)
    out[b,t,f] = sum_{d,e} x[b,t,d] * s[b,t,e] * cm[f,d,e]

Sharding: 8 cores, each owns 1024 contiguous positions of one batch row
(4 rows x 2 halves).  The scan carry across the half-split is recovered
exactly (to fp32) from a 256-position halo, since lam^256 ~ 4.5e-21 is far
below fp32 resolution.

Per-core dataflow (positions tiled 8 x 128):
  - carries: small PE matmuls build s(tile_start) for all 8 tiles at once
  - s tiles: triangular matmul  s = L @ x_tile + pow (x) carry   (PE, fp32)
  - main:    Y[p, (e,f)] = xT_tile.T @ W2   (PE, bf16 in / fp32 psum out)
    then the weighted e-reduction acc[p,f] += s[p,e] * Y[p,(e,f)] is split
    across three engine streams (per tile of 128 e-values):
      A (56 e): DVE scalar_tensor_tensor directly from PSUM, 2 alternating
        accumulators to keep the dependency chain off the critical path.
      B (40 e): Act (scalar engine) scaled-copies z_e = s_e*Y_e into bf16
        SBUF columns; DVE folds the 40 columns with a wide bf16 add-tree.
      C (32 e): Act copies Y chunks PSUM->SBUF; GpSimd does the STT there
        (GpSimd has no PSUM port).
  where W2[d, e*128+f] = cm[f, d, e]  (host-transposed, bf16).
"""

import numpy as np
import ml_dtypes

import concourse.bass as bass
import concourse.tile as tile
from concourse import bacc, mybir
from concourse.bass import ds, ts
from concourse.bass_utils import run_bass_kernel_spmd

B, S, D = 4, 2048, 128
NCORES = 8
CHUNK = S // 2          # positions per core (1024)
NT = CHUNK // 128       # position tiles per core (8)
P = 128
HALO = 256
F32 = mybir.dt.float32
BF16 = mybir.dt.bfloat16

# match the reference's fp32 constant 1.2 exactly
LAM = 1.0 / np.float64(np.float32(1.2))

# Every e produces one bf16 z column z_e = s_e * Y_e (z col index == e);
# a 7-level bf16 add-tree on DVE folds all 128 columns into the output.
# Producers per tile, by chunk kind (chunk = 4 consecutive e, f minor):
#   'C': Act copies Y chunk PSUM->SBUF, GpSimd broadcast-mult 4 e per op
#        (GpSimd has no PSUM port, ~3.3ns/elem, stream-bound)
#   'D' (consecutive pairs): one DVE broadcast-mult over a [P,8,128] 2-bank
#        PSUM mega tile = 8 e per op (amortizes DVE fixed cost)
#   'B': Act per-e scaled-copy straight from PSUM
# Mix tuned from engine-busy traces: DVE ~19us/tile (7 megas + tree),
# Act ~18us (16 z + 14 copies), GpSimd ~19us (14 chunks).
GROUP_KINDS = ["DDCB"] * 6 + ["DDCC"] * 2
ZB = 128

_CACHE = {}
LAST_RESULTS = None


def _host_constants():
    k = np.arange(P, dtype=np.float64)
    i = k
    # LT[i, k] = L[k, i] = lam^(k-i) for i < k   (lhsT of the triangular scan)
    LT = np.where(i[:, None] < k[None, :], LAM ** (k[None, :] - i[:, None]), 0.0)
    powv = (LAM ** k)[None, :]                      # [1, 128]
    vw = (LAM ** (P - i))[:, None]                  # [128, 1]
    j = np.arange(HALO, dtype=np.float64)           # halo weights lam^(256-j)
    hw = (LAM ** (HALO - j)).reshape(2, P).T        # [128, 2]  hw[i, u] = lam^(256-(u*128+i))
    # M9[t, jj]: c_t = sum_jj M9[t, jj] * V9[jj];  V9 = [c0, v_0..v_7]
    t = np.arange(NT, dtype=np.float64)
    M9 = np.zeros((NT, NT + 1), dtype=np.float64)
    M9[:, 0] = LAM ** (P * t)
    for tt in range(NT):
        for jj in range(tt):
            M9[tt, jj + 1] = LAM ** (P * (tt - 1 - jj))
    LT9 = M9.T                                      # [9, 8]
    f32 = np.float32
    return {
        "lt": LT.astype(f32),
        "powv": powv.astype(f32),
        "vw": vw.astype(f32),
        "hw": hw.astype(f32),
        "lt9": LT9.astype(f32),
    }


def _build_nc():
    nc = bacc.Bacc("TRN2", target_bir_lowering=False, debug=False,
                   num_devices=NCORES)
    x_d = nc.declare_dram_parameter("x", [P, NT, P], F32, isOutput=False)        # [i, t, e]
    xt_d = nc.declare_dram_parameter("xt", [P, CHUNK], BF16, isOutput=False)     # [d, p]
    halo_d = nc.declare_dram_parameter("halo", [P, 2, P], F32, isOutput=False)   # [i, u, e]
    w2_d = nc.declare_dram_parameter("w2", [P, P * P], BF16, isOutput=False)     # [d, (e,f)]
    lt_d = nc.declare_dram_parameter("lt", [P, P], F32, isOutput=False)
    pow_d = nc.declare_dram_parameter("powv", [1, P], F32, isOutput=False)
    vw_d = nc.declare_dram_parameter("vw", [P, 1], F32, isOutput=False)
    hw_d = nc.declare_dram_parameter("hw", [P, 2], F32, isOutput=False)
    lt9_d = nc.declare_dram_parameter("lt9", [NT + 1, NT], F32, isOutput=False)
    out_d = nc.declare_dram_parameter("out", [P, NT, P], F32, isOutput=True)  # [p, t, f]

    mult = mybir.AluOpType.mult
    add = mybir.AluOpType.add

    with tile.TileContext(nc) as tc:
        with tc.tile_pool(name="consts", bufs=1) as consts:
            w2_sb = [consts.tile([P, 2048], BF16, name=f"w2_sb{i}")
                     for i in range(8)]
            xt_sb = consts.tile([P, CHUNK], BF16)
            x_sb = consts.tile([P, NT, P], F32)
            halo_sb = consts.tile([P, 2, P], F32)
            lt_sb = consts.tile([P, P], F32)
            pow_sb = consts.tile([1, P], F32)
            vw_sb = consts.tile([P, 1], F32)
            hw_sb = consts.tile([P, 2], F32)
            lt9_sb = consts.tile([NT + 1, NT], F32)
            v9_sb = consts.tile([NT + 1, P], F32)
            c0_sb = consts.tile([1, P], F32)
            va_sb = consts.tile([1, 4 * P], F32)
            vb_sb = consts.tile([1, 4 * P], F32)
            c8_sb = consts.tile([NT, P], F32)
            c_all = consts.tile([1, NT * P], F32)    # [1, (t,e)] carries
            s_sb = consts.tile([P, NT, P], F32)      # [p, t, e]
            out_sb = consts.tile([P, NT, P], F32)    # [p, t, f]

            # small inputs first: the carry machinery (x, halo, consts) is on
            # the critical path to s_sb; the 4 MB w2 streams in behind it
            nc.sync.dma_start(out=x_sb[:, :, :], in_=x_d[:, :, :])
            nc.sync.dma_start(out=halo_sb[:, :, :], in_=halo_d[:, :, :])
            nc.sync.dma_start(out=lt_sb[:, :], in_=lt_d[:, :])
            nc.sync.dma_start(out=pow_sb[:, :], in_=pow_d[:, :])
            nc.sync.dma_start(out=vw_sb[:, :], in_=vw_d[:, :])
            nc.sync.dma_start(out=hw_sb[:, :], in_=hw_d[:, :])
            nc.sync.dma_start(out=lt9_sb[:, :], in_=lt9_d[:, :])
            nc.sync.dma_start(out=xt_sb[:, :], in_=xt_d[:, :])

            # ---- carries: c_t = s[tile_start t] for all 8 tiles ----
            with tc.tile_pool(name="psum_c", bufs=1, space="PSUM") as psum_c:
                c0_ps = psum_c.tile([1, P], F32)
                nc.tensor.matmul(c0_ps[:, :], lhsT=hw_sb[:, 0:1],
                                 rhs=halo_sb[:, 0, :], start=True, stop=False)
                nc.tensor.matmul(c0_ps[:, :], lhsT=hw_sb[:, 1:2],
                                 rhs=halo_sb[:, 1, :], start=False, stop=True)
                vps_a = psum_c.tile([1, 4 * P], F32, tag="vps_a")
                vps_b = psum_c.tile([1, 4 * P], F32, tag="vps_b")
                nc.tensor.matmul(vps_a[:, :], lhsT=vw_sb[:, :],
                                 rhs=x_sb[:, 0:4, :], start=True, stop=True)
                nc.tensor.matmul(vps_b[:, :], lhsT=vw_sb[:, :],
                                 rhs=x_sb[:, 4:8, :], start=True, stop=True)
                nc.scalar.copy(c0_sb[:, :], c0_ps[:, :])
                nc.scalar.copy(va_sb[:, :], vps_a[:, :])
                nc.scalar.copy(vb_sb[:, :], vps_b[:, :])
                nc.sync.dma_start(out=v9_sb[0:1, :], in_=c0_sb[:, :])
                nc.sync.dma_start(out=v9_sb[1:5, :], in_=va_sb[:, :])
                nc.sync.dma_start(out=v9_sb[5:9, :], in_=vb_sb[:, :])
                c_ps = psum_c.tile([NT, P], F32, tag="c_ps")
                nc.tensor.matmul(c_ps[:, :], lhsT=lt9_sb[:, :],
                                 rhs=v9_sb[:, :], start=True, stop=True)
                nc.scalar.copy(c8_sb[:, :], c_ps[:, :])
                nc.sync.dma_start(out=c_all[:, :], in_=c8_sb[:, :])

            # w2 now: its first segment lands well before the first
            # main-loop matmul needs it, and its descriptors no longer
            # delay the carry chain's small SBUF DMAs above
            for i in range(8):
                nc.sync.dma_start(out=w2_sb[i][:, :],
                                  in_=w2_d[:, ds(2048 * i, 2048)])

            # ---- s tiles: s = L @ x_t + pow (x) c_t ----
            with tc.tile_pool(name="psum_s", bufs=2, space="PSUM") as psum_s:
                for t in range(NT):
                    sp = psum_s.tile([P, P], F32)
                    nc.tensor.matmul(sp[:, :], lhsT=lt_sb[:, :],
                                     rhs=x_sb[:, t, :], start=True, stop=False)
                    c_rhs = c0_sb[:, :] if t == 0 else c_all[:, ts(t, P)]
                    nc.tensor.matmul(sp[:, :], lhsT=pow_sb[:, :],
                                     rhs=c_rhs, start=False, stop=True)
                    nc.scalar.copy(s_sb[:, t, :], sp[:, :])

            # ---- main: Y = xT_t.T @ W2 chunks; 3-stream weighted e-reduce ----
            with tc.tile_pool(name="psum_y", bufs=4, space="PSUM") as psum_y, \
                 tc.tile_pool(name="psum_m", bufs=2, space="PSUM") as psum_m, \
                 tc.tile_pool(name="zpool", bufs=3) as zpool, \
                 tc.tile_pool(name="ypool", bufs=6) as ypool:
                def tree_ops(t, zb):
                    # fold 128 bf16 z columns: 7 halving levels, last into
                    # f32, then DMA out -- as a list of thunks so the ops can
                    # be interleaved between the next tile's mega-mults
                    # (keeps DVE consuming psum_m so the PE never stalls)
                    ops = []
                    for half in (64, 32, 16, 8, 4, 2):
                        ops.append(lambda h=half: nc.vector.tensor_tensor(
                            zb[:, 0:h, :], zb[:, 0:h, :],
                            zb[:, h:2 * h, :], add))

                    def last():
                        nc.gpsimd.tensor_tensor(out_sb[:, t, :], zb[:, 0, :],
                                                zb[:, 1, :], add)
                        nc.sync.dma_start(out=out_d[:, t, :],
                                          in_=out_sb[:, t, :])
                    ops.append(last)
                    return ops

                def d_group(t, zb, g):
                    # the 'DD' pair of group g: DVE 8-e mega mult
                    xt_t = xt_sb[:, ts(t, P)]
                    i = GROUP_KINDS[g].index("D")
                    c = 4 * g + i
                    e0 = 4 * c
                    mp = psum_m.tile([P, 8, P], F32)
                    for h in range(2):
                        ch = c + h
                        nc.tensor.matmul(
                            mp[:, 4 * h:4 * h + 4, :], lhsT=xt_t,
                            rhs=w2_sb[ch // 4][:, ds(512 * (ch % 4), 512)],
                            start=True, stop=True)
                    nc.vector.tensor_tensor(
                        zb[:, e0:e0 + 8, :], mp[:, :, :],
                        s_sb[:, t, e0:e0 + 8, None].to_broadcast([P, 8, P]),
                        mult)

                def cb_group(t, zb, g):
                    # the 'C'/'B' chunks of group g
                    xt_t = xt_sb[:, ts(t, P)]
                    for i, kind in enumerate(GROUP_KINDS[g]):
                        if kind == "D":
                            continue
                        c = 4 * g + i
                        e0 = 4 * c
                        yp = psum_y.tile([P, 512], F32)
                        nc.tensor.matmul(
                            yp[:, :], lhsT=xt_t,
                            rhs=w2_sb[c // 4][:, ds(512 * (c % 4), 512)],
                            start=True, stop=True)
                        if kind == "C":
                            ysb = ypool.tile([P, 4, P], BF16)
                            nc.scalar.copy(ysb[:, :, :], yp[:, :])
                            nc.gpsimd.tensor_tensor(
                                zb[:, e0:e0 + 4, :], ysb[:, :, :],
                                s_sb[:, t, e0:e0 + 4, None].to_broadcast(
                                    [P, 4, P]),
                                mult)
                        else:  # 'B'
                            for jj in range(4):
                                e = e0 + jj
                                nc.scalar.mul(zb[:, e, :], yp[:, ts(jj, P)],
                                              s_sb[:, t, e:e + 1])

                # D-stream (PE pair-matmuls + DVE megas) runs one tile AHEAD
                # of the C/B streams, so the final tile leaves only Act and
                # GpSimd producer work overlapping the last trees on DVE.
                zbs = [None] * NT
                zbs[0] = zpool.tile([P, ZB, P], BF16, name="zb")
                for g in range(8):
                    d_group(0, zbs[0], g)
                pending = []
                for t in range(NT):
                    if t + 1 < NT:
                        zbs[t + 1] = zpool.tile([P, ZB, P], BF16,
                                                name="zb")
                    for g in range(8):
                        if pending:
                            pending.pop(0)()
                        if t + 1 < NT:
                            d_group(t + 1, zbs[t + 1], g)
                        cb_group(t, zbs[t], g)
                    for op in pending:
                        op()
                    pending = tree_ops(t, zbs[t])
                    zbs[t - 1] = None
                for op in pending:
                    op()
    nc.finalize()
    return nc


def _get_nc():
    if "nc" not in _CACHE:
        _CACHE["nc"] = _build_nc()
    return _CACHE["nc"]


def kernel(x, concept_map, _trace=False):
    global LAST_RESULTS
    x = np.asarray(x, dtype=np.float32)
    cm = np.asarray(concept_map, dtype=np.float32)
    assert x.shape == (B, S, D) and cm.shape == (D, D, D)

    consts = _host_constants()
    # W2[d, e*128+f] = cm[f, d, e]
    w2 = np.ascontiguousarray(
        np.transpose(cm, (1, 2, 0)).reshape(D, D * D)).astype(ml_dtypes.bfloat16)

    in_maps = []
    for core in range(NCORES):
        b, half = divmod(core, 2)
        lo = half * CHUNK
        xc = x[b, lo:lo + CHUNK]                          # [1024, 128]
        # [i, t, e] interleaved layout (partition = within-tile position)
        x_il = np.ascontiguousarray(
            xc.reshape(NT, P, D).transpose(1, 0, 2))
        xt = np.ascontiguousarray(xc.T).astype(ml_dtypes.bfloat16)  # [d, p]
        if half == 0:
            halo = np.zeros((P, 2, D), dtype=np.float32)
        else:
            h = x[b, lo - HALO:lo]                        # [256, 128]
            halo = np.ascontiguousarray(h.reshape(2, P, D).transpose(1, 0, 2))
        in_maps.append({
            "x": x_il, "xt": xt, "halo": halo, "w2": w2, **consts,
        })

    nc = _get_nc()
    res = run_bass_kernel_spmd(nc, in_maps, list(range(NCORES)), trace=_trace)
    LAST_RESULTS = res

    out = np.empty((B, S, D), dtype=np.float32)
    for core in range(NCORES):
        b, half = divmod(core, 2)
        o = res.results[core]["out"]                      # [p, t, f]
        out[b, half * CHUNK:(half + 1) * CHUNK] = (
            o.transpose(1, 0, 2).reshape(CHUNK, D))
    return out
